# revision 54
# baseline (speedup 1.0000x reference)
"""Trainium2 Bass kernel for the EntangledInterferenceLayer problem.

Math transformations done on host (numpy), all exact up to fp rounding:
  * The HxH entanglement mix commutes with RoPE (cos/sin are head-independent),
    so it folds into the Q/K projection weights + biases.
  * The per-head phase shift rotates q and k by the same complex phase, and the
    attention logits use q * conj(k) -> the phase cancels exactly.  Dropped.
  * 1/sqrt(head_dim) folds into the Q weights/bias.
  * The V-projection bias contributes bv @ Wo to every output row (softmax rows
    sum to 1), so it folds into the output bias.

Sharding (8 cores): core = (batch b, head-group g of 4 heads). Each core
projects Q/K/V for its heads, runs causal complex-magnitude attention, then an
AllGather of attention outputs within the 4-core batch group lets every core
compute a 256-column slice of both output projections.

Device-side design (v2, bf16):
  * All matmul operands bf16 (2x stream rate vs fp32r at the ramped PE clock);
    PSUM accumulation stays f32.  Weights and x are pre-tiled on host into
    SBUF layout [128, kt, m] so every load is one fat DMA, loaded once.
  * Q/K computed transposed ([complex-component-row, token]); per head the 128
    contraction rows are [qr-rot, qr-nr, qi-rot, qi-nr]; K2 = [-ki, kr] gives
    imaginary logits with plain matmuls.
  * Projection biases are folded into the PSUM->SBUF evacuation via the ACT
    engine's per-partition bias operand (no bias matmuls).
  * Scores are built [kv, q] with causally-trimmed streams, packed contiguously
    per (chunk, head) so Sqrt and Exp run as one big ACT op each (table loads
    batched pairwise across heads).
  * AV uses the score tile as the *stationary* operand producing [token, d]
    blocks; V real/imag/ones are one [128,129] moving operand, so softmax
    denominators land per-partition: reciprocal is a [128,1] DVE op and the
    normalisation folds into a single strided tensor_scalar_mul.
  * Normalised outputs are transposed back to [head-dim, token] with PE
    transposes (identity matmul), staged to DRAM, and AllGathered per chunk so
    the collective overlaps the next chunk's attention / output projection.
"""

import math

import numpy as np

B, S, DIM = 2, 1024, 1024
HEADS, HD, ROTD = 16, 64, 32
GH = 4  # heads per core
ODC = 256  # out-dim columns per core
NCORES = 8

_PAIRSWAP = [i ^ 1 for i in range(32)]


def _register_dve_op(name, spec_builder):
    """Register a fused custom DVE op (idempotent)."""
    from concourse import dve_ops as DO
    from concourse.dve_spec import lower

    if name in DO._SUB_OPCODE_FOR_NAME:
        return next(o for o in DO.OPS if o.name == name)
    spec = spec_builder()
    opcode = DO._CUSTOM_DVE_ROW_BASE + len(DO.OPS)
    DO._SUB_OPCODE_FOR_NAME[name] = opcode
    shas = {}
    for ver in ("v3", "v4"):
        try:
            s = DO.DveOpSpec(
                name=name, opcode=opcode, uops=lower(spec, ver=ver), rd1_en=True
            )
            shas[ver] = s.sha(ver)
        except Exception:
            pass
    op = DO.DveOp(name, spec, subdim=False, uops_sha=shas)
    DO.OPS.append(op)
    DO.CUSTOM_DVE_SPECS[name] = spec
    return op


def _register_magsq():
    """out = (in0^2 + in1^2) * imm2"""
    import numpy as np
    from concourse.dve_spec import Spec, Src0, Src1, C2, sq

    return _register_dve_op(
        "ANT_MAGSQ",
        lambda: Spec(
            body=(sq(Src0) + sq(Src1)) * C2,
            reference=lambda in0, in1, s0, s1, imm2: (
                in0.astype(np.float32) ** 2 + in1.astype(np.float32) ** 2
            )
            * np.float32(imm2),
        ),
    )


def _register_sqadd():
    """out = (in0^2 + in1) * imm2 — in1 is a pre-squared operand."""
    import numpy as np
    from concourse.dve_spec import Spec, Src0, Src1, C2, sq

    return _register_dve_op(
        "ANT_SQADD",
        lambda: Spec(
            body=(sq(Src0) + Src1) * C2,
            reference=lambda in0, in1, s0, s1, imm2: (
                in0.astype(np.float32) ** 2 + in1.astype(np.float32)
            )
            * np.float32(imm2),
        ),
    )


def _build(gt: float, groups=None):
    import concourse.mybir as mybir
    import concourse.tile as tile
    from concourse import bacc

    f32 = mybir.dt.float32
    bf16 = mybir.dt.bfloat16
    AF = mybir.ActivationFunctionType
    magsq = _register_magsq()
    sqadd = _register_sqadd()

    nc = bacc.Bacc("TRN2", target_bir_lowering=False, num_devices=NCORES)
    if groups is None:
        groups = [[0, 1, 2, 3], [4, 5, 6, 7]]

    # host-pre-tiled inputs: [128, kt*m] so DMAs are one fat line per partition
    xr = nc.dram_tensor("xr", [128, 8 * S], bf16, kind="ExternalInput")
    xi = nc.dram_tensor("xi", [128, 8 * S], bf16, kind="ExternalInput")
    w = {
        nm: nc.dram_tensor(nm, [128, 8 * 256], bf16, kind="ExternalInput")
        for nm in ["wqr", "wqi", "wkr", "wki", "wor", "woi"]
    }
    wv = nc.dram_tensor("wv", [128, 8 * 512], bf16, kind="ExternalInput")
    bqk_d = nc.dram_tensor("bqk", [128, 8], f32, kind="ExternalInput")
    bo_d = nc.dram_tensor("bo", [128, 4], f32, kind="ExternalInput")
    cosd = nc.dram_tensor("cosd", [128, S], bf16, kind="ExternalInput")
    sind = nc.dram_tensor("sind", [128, S], bf16, kind="ExternalInput")
    identd = nc.dram_tensor("identd", [128, 128], bf16, kind="ExternalInput")
    o_r = nc.dram_tensor("o_r", [ODC, S], bf16, kind="ExternalOutput")
    o_i = nc.dram_tensor("o_i", [ODC, S], bf16, kind="ExternalOutput")

    def mm(out, lhsT, rhs, start, stop):
        nc.tensor.matmul(out, lhsT=lhsT, rhs=rhs, start=start, stop=stop)

    # packed score-column offsets per chunk: pk[qc][kvt], total PK[qc]
    pk, PK = [], []
    for qc in range(2):
        offs, run = [], 0
        for kvt in range((qc + 1) * 4):
            offs.append(run)
            lo = max(0, (kvt - 4 * qc) * 128)
            run += 512 - lo
        pk.append(offs)
        PK.append(run)

    with tile.TileContext(nc) as tc:
        with (
            tc.tile_pool(name="consts", bufs=1) as consts,
            tc.tile_pool(name="stage", bufs=3) as stage,
            tc.tile_pool(name="evp", bufs=3) as evp,
            tc.tile_pool(name="c1p", bufs=2) as c1p,
            tc.tile_pool(name="efp", bufs=4) as efp,
            tc.tile_pool(name="ebp", bufs=4) as ebp,
            tc.tile_pool(name="recp", bufs=4) as recp,
            tc.tile_pool(name="atp", bufs=8) as atp,
            tc.tile_pool(name="ttp", bufs=4) as ttp,
            tc.tile_pool(name="lop", bufs=4) as lop,
            tc.tile_pool(name="oop", bufs=2) as oop,
            tc.tile_pool(name="psA", bufs=3, space="PSUM") as psA,
            tc.tile_pool(name="psB", bufs=2, space="PSUM") as psB,
            tc.tile_pool(name="dram", bufs=1, space="DRAM") as dram,
        ):
            # ---- constants / persistent SBUF ----
            # x chunk-0 and the QK weights first: they gate the first matmul.
            x_sb = {}
            for key, src in (("r", xr), ("i", xi)):
                t = consts.tile([128, 8, S], bf16, tag=f"x{key}")
                rr = src.rearrange("p (kt m) -> p kt m", kt=8)
                nc.sync.dma_start(t[:, :, 0:512], rr[:, :, 0:512])
                x_sb[key] = t
            w_sb = {}
            for nm in ["wqr", "wqi", "wkr", "wki"]:
                t = consts.tile([128, 8, 256], bf16, tag=nm)
                nc.sync.dma_start(t, w[nm].rearrange("p (kt m) -> p kt m", kt=8))
                w_sb[nm] = t
            bqk_sb = consts.tile([128, 8], f32, tag="bqk")
            nc.sync.dma_start(bqk_sb, bqk_d[:, :])
            cos_sb = consts.tile([128, S], bf16, tag="cos")
            nc.sync.dma_start(cos_sb, cosd[:, :])
            sin_sb = consts.tile([128, S], bf16, tag="sin")
            nc.sync.dma_start(sin_sb, sind[:, :])
            wv_sb = consts.tile([128, 8, 512], bf16, tag="wv")
            nc.sync.dma_start(wv_sb, wv.rearrange("p (kt m) -> p kt m", kt=8))
            for key, src in (("r", xr), ("i", xi)):
                rr = src.rearrange("p (kt m) -> p kt m", kt=8)
                nc.sync.dma_start(x_sb[key][:, :, 512:1024], rr[:, :, 512:1024])

            # tiny warmup collective: absorbs the CC core's first-collective
            # startup cost (~11us) while the projections run.
            wagin = dram.tile([1, 4], f32, tag="wagin", name="wagin")
            wagout = dram.tile([4, 1, 4], f32, tag="wagout", name="wagout")
            wz = consts.tile([1, 4], f32, tag="wz")
            nc.vector.memset(wz, 0.0)
            nc.gpsimd.dma_start(wagin[:, :], wz)
            nc.gpsimd.collective_compute(
                "AllGather",
                mybir.AluOpType.bypass,
                replica_groups=groups,
                ins=[wagin[:].opt()],
                outs=[wagout[:].opt()],
            )

            eps_t = consts.tile([128, 1], f32, tag="eps")
            nc.vector.memset(eps_t, 1e-6 * float(gt) * float(gt))
            ident = consts.tile([128, 128], bf16, tag="ident")
            nc.sync.dma_start(ident, identd[:, :])
            bo_sb = consts.tile([128, 4], f32, tag="bo")
            nc.sync.dma_start(bo_sb, bo_d[:, :])
            for nm in ["wor", "woi"]:
                t = consts.tile([128, 8, 256], bf16, tag=nm)
                nc.sync.dma_start(t, w[nm].rearrange("p (kt m) -> p kt m", kt=8))
                w_sb[nm] = t

            Q = consts.tile([128, GH, S], bf16, tag="Q")
            K1 = consts.tile([128, GH, S], bf16, tag="K1")
            K2 = consts.tile([128, GH, S], bf16, tag="K2")
            # V combined per kv-tile/head: cols 0:64 = vr, 64:128 = vi, 128 = ones
            Vc = consts.tile([128, 8, GH, 129], bf16, tag="Vc")
            nc.vector.memset(Vc[:, :, :, 128:129], 1.0)

            # ---- phase 1: projections ----
            # (name, x key, w name, bias col base, rot targets, nr targets)
            # targets: (tensor, row0); ki additionally writes negated K2 rows.
            projs = [
                ("qr", "r", "wqr", 0, [(0, 0)], [(0, 32)]),
                ("qi", "i", "wqi", 2, [(0, 64)], [(0, 96)]),
                ("kr", "r", "wkr", 4, [(1, 0), (2, 64)], [(1, 32), (2, 96)]),
                ("ki", "i", "wki", 6, [(1, 64)], [(1, 96)]),
            ]
            qk_tensors = {0: Q, 1: K1, 2: K2}

            agin = [
                dram.tile([512, 512], bf16, tag=f"agin{qc}", name=f"agin{qc}")
                for qc in range(2)
            ]
            agout = [
                [
                    dram.tile(
                        [4, 256, 512], bf16, tag=f"agout{qc}_{ri}",
                        name=f"agout{qc}_{ri}",
                    )
                    for ri in range(2)
                ]
                for qc in range(2)
            ]
            gg = float(gt) * float(gt)

            def scores(qc, h, ef, spare=False):
                """Score matmuls (kv tiles paired into 2-bank PSUM tiles) +
                one psi evacuation + one fused |z|^2 per pair.  The psi
                evacuation alternates between the scalar engine (as a
                table-free Square, feeding (a^2+b)*s) and the vector engine
                (plain copy, feeding (a^2+b^2)*s) to balance engine load.
                With spare=True, uses single-bank tiles from the psB pool so
                the scores can overlap projections still running on psA."""
                nkv = (qc + 1) * 4
                step = 1 if spare else 2
                for kv0 in range(0, nkv, step):
                    if spare:
                        psr2 = psB.tile([128, 512], f32, tag="pav")
                        psi2 = psB.tile([128, 512], f32, tag="pav")
                    else:
                        psr2 = psA.tile([128, 1024], f32, tag="ps2")
                        psi2 = psA.tile([128, 1024], f32, tag="ps2")
                    col = 0
                    for kvt in range(kv0, kv0 + step):
                        lo = max(0, (kvt - 4 * qc) * 128)
                        N = 512 - lo
                        qsl = Q[:, h, qc * 512 + lo : (qc + 1) * 512]
                        ksl = slice(kvt * 128, (kvt + 1) * 128)
                        mm(
                            psr2[:, col : col + N], K1[:, h, ksl], qsl,
                            start=True, stop=True,
                        )
                        mm(
                            psi2[:, col : col + N], K2[:, h, ksl], qsl,
                            start=True, stop=True,
                        )
                        col += N
                    c1 = c1p.tile([128, 1024], f32, tag="c1")
                    if (kv0 // 2) % 2 == 0:
                        nc.scalar.activation(c1[:, :col], psi2[:, :col], AF.Square)
                        op, imm = sqadd, gg
                    else:
                        nc.vector.tensor_copy(c1[:, :col], psi2[:, :col])
                        op, imm = magsq, gg
                    nc.vector._custom_dve(
                        op,
                        out=ef[:, pk[qc][kv0] : pk[qc][kv0] + col],
                        in0=psr2[:, :col],
                        in1=c1[:, :col],
                        imm2=imm,
                    )

            def av(qc, h, eb, A):
                """AV with stationary score blocks -> [token, d] + normalise."""
                for t in range(4):
                    nkv_t = 4 * qc + t + 1
                    pd = psB.tile([128, 512], f32, tag="pav")
                    for kvt in range(nkv_t):
                        lo = max(0, (kvt - 4 * qc) * 128)
                        blk = pk[qc][kvt] + t * 128 - lo
                        mm(
                            pd[:, 0:129],
                            eb[:, blk : blk + 128],
                            Vc[:, kvt, h, :],
                            start=(kvt == 0),
                            stop=(kvt == nkv_t - 1),
                        )
                    rec = recp.tile([128, 1], f32, tag="rec")
                    nc.vector.reciprocal(rec, pd[:, 128:129])
                    # r cols -> A[t][:, h*64:...], i cols -> A[t][:, 256+h*64:...]
                    av_view = pd[:, 0:128].rearrange("p (x d) -> p x d", x=2)
                    out_view = A[t].rearrange("p (x hd) -> p x hd", x=2)[
                        :, :, h * 64 : (h + 1) * 64
                    ]
                    nc.vector.tensor_scalar_mul(out_view, av_view, rec)

            def attention(qc, spare=False):
                A = [
                    atp.tile([128, 512], bf16, tag="A", name=f"A{qc}_{t}")
                    for t in range(4)
                ]
                efs, ebs = {}, {}
                for h in range(GH):
                    efs[h] = efp.tile([128, PK[1]], bf16, tag="ef", name=f"ef{qc}_{h}")
                    scores(qc, h, efs[h], spare=spare)
                for h in range(GH):
                    nc.scalar.activation(
                        efs[h][:, : PK[qc]],
                        efs[h][:, : PK[qc]],
                        AF.Sqrt,
                        bias=eps_t,
                    )
                for h in range(GH):
                    ebs[h] = ebp.tile([128, PK[1]], bf16, tag="eb", name=f"eb{qc}_{h}")
                    nc.scalar.activation(
                        ebs[h][:, : PK[qc]], efs[h][:, : PK[qc]], AF.Exp
                    )
                    for kvt in range(qc * 4, (qc + 1) * 4):
                        po = pk[qc][kvt]
                        nc.gpsimd.affine_select(
                            out=ebs[h][:, po : po + 128],
                            in_=ebs[h][:, po : po + 128],
                            compare_op=mybir.AluOpType.is_ge,
                            fill=0.0,
                            base=0,
                            channel_multiplier=-1,
                            pattern=[[1, 128]],
                        )

                # AVs for a head pair, then immediately transpose + stage +
                # AllGather that pair's rows (r and i) while the next pair's
                # AVs run.  agin rows: [r-h01 | i-h01 | r-h23 | i-h23].
                for hp in range(2):
                    av(qc, 2 * hp, ebs[2 * hp], A)
                    av(qc, 2 * hp + 1, ebs[2 * hp + 1], A)
                    for j, blk in enumerate((hp, 2 + hp)):
                        tb2 = ttp.tile(
                            [128, 512], bf16, tag="tb2", name=f"tb{qc}_{hp}_{j}"
                        )
                        for t in range(4):
                            pt = psB.tile([128, 1024], bf16, tag="pav")
                            nc.tensor.transpose(
                                pt[:, 0:128],
                                A[t][:, blk * 128 : (blk + 1) * 128],
                                ident,
                            )
                            nc.vector.tensor_copy(
                                tb2[:, t * 128 : (t + 1) * 128], pt[:, 0:128]
                            )
                        nc.gpsimd.dma_start(
                            agin[qc][hp * 256 + j * 128 : hp * 256 + (j + 1) * 128, :],
                            tb2,
                        )
                    rs = slice(hp * 256, (hp + 1) * 256)
                    nc.gpsimd.collective_compute(
                        "AllGather",
                        mybir.AluOpType.bypass,
                        replica_groups=groups,
                        ins=[agin[qc][rs, :].opt()],
                        outs=[agout[qc][hp][:].opt()],
                    )

            for c in range(2):
                csl = slice(c * 512, (c + 1) * 512)
                for pname, xkey, wname, bc, rot_tgts, nr_tgts in projs:
                    for mt in range(2):  # 0 = rot dims, 1 = non-rot dims
                        pst = psA.tile([128, 512], f32, tag="ps2")
                        for kt in range(8):
                            mm(
                                pst,
                                w_sb[wname][:, kt, mt * 128 : (mt + 1) * 128],
                                x_sb[xkey][:, kt, csl],
                                start=(kt == 0),
                                stop=(kt == 7),
                            )
                        bcol = bqk_sb[:, bc + mt : bc + mt + 1]
                        if mt == 0:
                            # bias-add during evacuation, then rope in bf16
                            tb = stage.tile([128, 512], bf16, tag="tb")
                            nc.scalar.activation(tb, pst, AF.Identity, bias=bcol)
                            shuf = stage.tile([128, 512], bf16, tag="shuf")
                            nc.vector.stream_shuffle(shuf, tb, mask=_PAIRSWAP)
                            nc.vector.tensor_mul(shuf, shuf, sin_sb[:, csl])
                            t2 = stage.tile([128, 512], bf16, tag="t2")
                            nc.vector.tensor_mul(t2, tb, cos_sb[:, csl])
                            nc.vector.tensor_add(t2, t2, shuf)
                            src_t = t2
                        else:
                            evn = evp.tile([128, 512], bf16, tag="ev")
                            nc.scalar.activation(evn, pst, AF.Identity, bias=bcol)
                            src_t = evn
                        tgts = rot_tgts if mt == 0 else nr_tgts
                        eng = nc.sync if mt == 0 else nc.scalar
                        for tid, row0 in tgts:
                            dst = qk_tensors[tid]
                            for h in range(GH):
                                eng.dma_start(
                                    dst[row0 : row0 + 32, h, csl],
                                    src_t[h * 32 : (h + 1) * 32, :],
                                )
                        if pname == "ki":  # negated copy into K2 rows 0:32/32:64
                            neg = evp.tile([128, 512], bf16, tag="ev")
                            nc.vector.tensor_scalar_mul(neg, src_t, -1.0)
                            row0 = 0 if mt == 0 else 32
                            for h in range(GH):
                                nc.sync.dma_start(
                                    K2[row0 : row0 + 32, h, csl],
                                    neg[h * 32 : (h + 1) * 32, :],
                                )

                # V: stationary x-block, moving [wvr|wvi] columns
                for tl in range(4):
                    tt = c * 4 + tl
                    tsl = slice(c * 512 + tl * 128, c * 512 + (tl + 1) * 128)
                    pv = psA.tile([128, 512], f32, tag="ps2")
                    for kt in range(8):
                        mm(
                            pv[:, 0:256],
                            x_sb["r"][:, kt, tsl],
                            wv_sb[:, kt, 0:256],
                            start=(kt == 0),
                            stop=(kt == 7),
                        )
                    for kt in range(8):
                        mm(
                            pv[:, 256:512],
                            x_sb["i"][:, kt, tsl],
                            wv_sb[:, kt, 256:512],
                            start=(kt == 0),
                            stop=(kt == 7),
                        )
                    ov = evp.tile([128, 512], bf16, tag="ov")
                    nc.scalar.copy(ov, pv)
                    nc.scalar.dma_start(
                        Vc[:, tt, :, 0:64],
                        ov[:, 0:256].rearrange("p (h d) -> p h d", h=GH),
                    )
                    nc.scalar.dma_start(
                        Vc[:, tt, :, 64:128],
                        ov[:, 256:512].rearrange("p (h d) -> p h d", h=GH),
                    )

                if c == 0:
                    # chunk-0 attention only needs chunk-0 projections; its
                    # scores use the spare psB banks so they overlap chunk-1
                    # projections without fighting for the psA score slots.
                    attention(0, spare=True)

            attention(1)

            # ---- phase 3: output projections (AG(1) hides under O-proj(0)) ----
            for qc in range(2):
                for ri, wname, odst in ((0, "wor", o_r), (1, "woi", o_i)):
                    pos = [
                        psA.tile([128, 512], f32, tag="ps2", name=f"po{_i}")
                        for _i in range(2)
                    ]
                    for ht in range(8):
                        g, hp = ht // 2, ht % 2
                        lt = lop.tile([128, 512], bf16, tag="lt")
                        nc.sync.dma_start(
                            lt, agout[qc][hp][g, ri * 128 : (ri + 1) * 128, :]
                        )
                        for odt in range(2):
                            mm(
                                pos[odt],
                                w_sb[wname][:, ht, odt * 128 : (odt + 1) * 128],
                                lt,
                                start=(ht == 0),
                                stop=(ht == 7),
                            )
                    for odt in range(2):
                        oo = oop.tile([128, 512], bf16, tag="oo")
                        nc.scalar.activation(
                            oo,
                            pos[odt],
                            AF.Identity,
                            bias=bo_sb[:, 2 * ri + odt : 2 * ri + odt + 1],
                        )
                        nc.scalar.dma_start(
                            odst[
                                odt * 128 : (odt + 1) * 128,
                                qc * 512 : (qc + 1) * 512,
                            ],
                            oo,
                        )

    return nc


def _host_prep(inputs):
    """Fold ent/scale/bv on host; build per-core input maps (bf16 device layout)."""
    import ml_dtypes

    bf16 = ml_dtypes.bfloat16
    f = lambda x: np.asarray(x, dtype=np.float32)
    real, imag = f(inputs["real"]), f(inputs["imag"])
    ent = np.asarray(inputs["ent"], np.float64)
    scale = 1.0 / math.sqrt(HD)

    def fold_w(W, do_ent, sc=1.0):
        W = np.asarray(W, np.float64).reshape(DIM, HEADS, HD)
        if do_ent:
            W = np.einsum("chd,hx->cxd", W, ent)
        return W * sc  # [DIM, HEADS, HD] float64

    def fold_b(b, do_ent, sc=1.0):
        b = np.asarray(b, np.float64).reshape(HEADS, HD)
        if do_ent:
            b = np.einsum("hd,hx->xd", b, ent)
        return b * sc

    Wq_r = fold_w(inputs["Wq_r"], True, scale)
    Wq_i = fold_w(inputs["Wq_i"], True, scale)
    Wk_r = fold_w(inputs["Wk_r"], True)
    Wk_i = fold_w(inputs["Wk_i"], True)
    Wv_r = fold_w(inputs["Wv_r"], False)
    Wv_i = fold_w(inputs["Wv_i"], False)
    bq_r = fold_b(inputs["bq_r"], True, scale)
    bq_i = fold_b(inputs["bq_i"], True, scale)
    bk_r = fold_b(inputs["bk_r"], True)
    bk_i = fold_b(inputs["bk_i"], True)
    Wo_r = np.asarray(inputs["Wo_r"], np.float64)
    Wo_i = np.asarray(inputs["Wo_i"], np.float64)
    bo_r = np.asarray(inputs["bo_r"], np.float64) + np.asarray(
        inputs["bv_r"], np.float64
    ) @ Wo_r
    bo_i = np.asarray(inputs["bo_i"], np.float64) + np.asarray(
        inputs["bv_i"], np.float64
    ) @ Wo_i

    strength = float(np.asarray(inputs["strength"]).reshape(-1)[0])
    temp = float(np.asarray(inputs["temp"]).reshape(-1)[0])
    gt = (1.0 / (1.0 + math.exp(-strength))) / max(temp, 0.01)

    # rope tables in device layout: row h*32+d (d<32), freq j=d//2
    rot_freqs = np.asarray(inputs["rot_freqs"], np.float64)  # [16]
    pos = np.arange(S, dtype=np.float64)
    emb = pos[:, None] * rot_freqs[None, :]  # [S, 16]
    cos_t = np.cos(emb)
    sin_t = np.sin(emb)
    cosd = np.empty((128, S), np.float32)
    sind = np.empty((128, S), np.float32)
    for hh in range(4):
        for d in range(32):
            r = hh * 32 + d
            cosd[r] = cos_t[:, d // 2]
            sind[r] = (-sin_t if d % 2 == 0 else sin_t)[:, d // 2]

    def tile_kp(Wdev):
        # [DIM, M] -> [128, 8*M] with (kp, kt, m) = W[kt*128+kp, m]
        M = Wdev.shape[1]
        return np.ascontiguousarray(
            Wdev.reshape(8, 128, M).transpose(1, 0, 2).reshape(128, 8 * M)
        )

    def qk_dev(Wf, bf_, g):
        # [DIM,H,HD]/[H,HD] -> per-core [DIM,256]/[256] in [rot x 4h | nr x 4h]
        hs = slice(g * GH, (g + 1) * GH)
        Wc, bc = Wf[:, hs, :], bf_[hs, :]
        wd = np.concatenate(
            [
                Wc[:, :, :ROTD].reshape(DIM, GH * ROTD),
                Wc[:, :, ROTD:].reshape(DIM, GH * ROTD),
            ],
            axis=1,
        )
        bd = np.concatenate(
            [bc[:, :ROTD].reshape(GH * ROTD), bc[:, ROTD:].reshape(GH * ROTD)]
        )
        return wd, bd

    ident = np.eye(128, dtype=np.float32)

    in_maps = []
    for core in range(NCORES):
        b, g = core // 4, core % 4
        hs = slice(g * GH, (g + 1) * GH)
        xr_dev = np.ascontiguousarray(
            real[b].T.reshape(8, 128, S).transpose(1, 0, 2).reshape(128, 8 * S)
        )
        xi_dev = np.ascontiguousarray(
            imag[b].T.reshape(8, 128, S).transpose(1, 0, 2).reshape(128, 8 * S)
        )
        wv_dev = np.concatenate(
            [Wv_r[:, hs, :].reshape(DIM, 256), Wv_i[:, hs, :].reshape(DIM, 256)],
            axis=1,
        )
        bqk = np.empty((128, 8), np.float32)
        m = {
            "xr": xr_dev.astype(bf16),
            "xi": xi_dev.astype(bf16),
            "cosd": cosd.astype(bf16),
            "sind": sind.astype(bf16),
            "identd": ident.astype(bf16),
            "wv": tile_kp(wv_dev).astype(bf16),
            "wor": tile_kp(Wo_r[:, g * ODC : (g + 1) * ODC]).astype(bf16),
            "woi": tile_kp(Wo_i[:, g * ODC : (g + 1) * ODC]).astype(bf16),
        }
        for j, (nm, Wf, bf_) in enumerate(
            (
                ("qr", Wq_r, bq_r),
                ("qi", Wq_i, bq_i),
                ("kr", Wk_r, bk_r),
                ("ki", Wk_i, bk_i),
            )
        ):
            wd, bd = qk_dev(Wf, bf_, g)
            m["w" + nm] = tile_kp(wd).astype(bf16)
            bqk[:, 2 * j] = bd[0:128]
            bqk[:, 2 * j + 1] = bd[128:256]
        m["bqk"] = bqk
        bo = np.empty((128, 4), np.float32)
        bo[:, 0] = bo_r[g * ODC : g * ODC + 128]
        bo[:, 1] = bo_r[g * ODC + 128 : g * ODC + 256]
        bo[:, 2] = bo_i[g * ODC : g * ODC + 128]
        bo[:, 3] = bo_i[g * ODC + 128 : g * ODC + 256]
        m["bo"] = bo
        in_maps.append(m)
    return in_maps, gt


def kernel(**inputs):
    from concourse import bass_utils

    in_maps, gt = _host_prep(inputs)
    nc = _build(gt)
    nc.finalize()
    res = bass_utils.run_bass_kernel_spmd(nc, in_maps, core_ids=list(range(NCORES)))
    out_r = np.empty((B, S, DIM), np.float32)
    out_i = np.empty((B, S, DIM), np.float32)
    for core in range(NCORES):
        b, g = core // 4, core % 4
        out_r[b, :, g * ODC : (g + 1) * ODC] = (
            np.asarray(res.results[core]["o_r"]).astype(np.float32).T
        )
        out_i[b, :, g * ODC : (g + 1) * ODC] = (
            np.asarray(res.results[core]["o_i"]).astype(np.float32).T
        )
    return np.stack([out_r, out_i], axis=0)


# revision 55
# speedup vs baseline: 1.0902x; 1.0902x over previous
"""Trainium2 Bass kernel for the EntangledInterferenceLayer problem.

Math transformations done on host (numpy), all exact up to fp rounding:
  * The HxH entanglement mix commutes with RoPE (cos/sin are head-independent),
    so it folds into the Q/K projection weights + biases.
  * The per-head phase shift rotates q and k by the same complex phase, and the
    attention logits use q * conj(k) -> the phase cancels exactly.  Dropped.
  * 1/sqrt(head_dim) folds into the Q weights/bias.
  * The V-projection bias contributes bv @ Wo to every output row (softmax rows
    sum to 1), so it folds into the output bias.

Sharding (8 cores): core = (batch b, head-group g of 4 heads). Each core
projects Q/K/V for its heads, runs causal complex-magnitude attention, then an
AllGather of attention outputs within the 4-core batch group lets every core
compute a 256-column slice of both output projections.

Device-side design (v2, bf16):
  * All matmul operands bf16 (2x stream rate vs fp32r at the ramped PE clock);
    PSUM accumulation stays f32.  Weights and x are pre-tiled on host into
    SBUF layout [128, kt, m] so every load is one fat DMA, loaded once.
  * Q/K computed transposed ([complex-component-row, token]); per head the 128
    contraction rows are [qr-rot, qr-nr, qi-rot, qi-nr]; K2 = [-ki, kr] gives
    imaginary logits with plain matmuls.
  * Projection biases are folded into the PSUM->SBUF evacuation via the ACT
    engine's per-partition bias operand (no bias matmuls).
  * Scores are built [kv, q] with causally-trimmed streams, packed contiguously
    per (chunk, head) so Sqrt and Exp run as one big ACT op each (table loads
    batched pairwise across heads).
  * AV uses the score tile as the *stationary* operand producing [token, d]
    blocks; V real/imag/ones are one [128,129] moving operand, so softmax
    denominators land per-partition: reciprocal is a [128,1] DVE op and the
    normalisation folds into a single strided tensor_scalar_mul.
  * Normalised outputs are transposed back to [head-dim, token] with PE
    transposes (identity matmul), staged to DRAM, and AllGathered per chunk so
    the collective overlaps the next chunk's attention / output projection.
"""

import math

import numpy as np

B, S, DIM = 2, 1024, 1024
HEADS, HD, ROTD = 16, 64, 32
GH = 4  # heads per core
ODC = 256  # out-dim columns per core
NCORES = 8

_PAIRSWAP = [i ^ 1 for i in range(32)]


def _register_dve_op(name, spec_builder):
    """Register a fused custom DVE op (idempotent)."""
    from concourse import dve_ops as DO
    from concourse.dve_spec import lower

    if name in DO._SUB_OPCODE_FOR_NAME:
        return next(o for o in DO.OPS if o.name == name)
    spec = spec_builder()
    opcode = DO._CUSTOM_DVE_ROW_BASE + len(DO.OPS)
    DO._SUB_OPCODE_FOR_NAME[name] = opcode
    shas = {}
    for ver in ("v3", "v4"):
        try:
            s = DO.DveOpSpec(
                name=name, opcode=opcode, uops=lower(spec, ver=ver), rd1_en=True
            )
            shas[ver] = s.sha(ver)
        except Exception:
            pass
    op = DO.DveOp(name, spec, subdim=False, uops_sha=shas)
    DO.OPS.append(op)
    DO.CUSTOM_DVE_SPECS[name] = spec
    return op


def _register_magsq():
    """out = (in0^2 + in1^2) * imm2"""
    import numpy as np
    from concourse.dve_spec import Spec, Src0, Src1, C2, sq

    return _register_dve_op(
        "ANT_MAGSQ",
        lambda: Spec(
            body=(sq(Src0) + sq(Src1)) * C2,
            reference=lambda in0, in1, s0, s1, imm2: (
                in0.astype(np.float32) ** 2 + in1.astype(np.float32) ** 2
            )
            * np.float32(imm2),
        ),
    )


def _register_sqadd():
    """out = (in0^2 + in1) * imm2 — in1 is a pre-squared operand."""
    import numpy as np
    from concourse.dve_spec import Spec, Src0, Src1, C2, sq

    return _register_dve_op(
        "ANT_SQADD",
        lambda: Spec(
            body=(sq(Src0) + Src1) * C2,
            reference=lambda in0, in1, s0, s1, imm2: (
                in0.astype(np.float32) ** 2 + in1.astype(np.float32)
            )
            * np.float32(imm2),
        ),
    )


def _build(gt: float, groups=None):
    import concourse.mybir as mybir
    import concourse.tile as tile
    from concourse import bacc

    f32 = mybir.dt.float32
    bf16 = mybir.dt.bfloat16
    AF = mybir.ActivationFunctionType
    magsq = _register_magsq()
    sqadd = _register_sqadd()

    nc = bacc.Bacc("TRN2", target_bir_lowering=False, num_devices=NCORES)
    if groups is None:
        groups = [[0, 1, 2, 3], [4, 5, 6, 7]]

    # host-pre-tiled inputs: [128, kt*m] so DMAs are one fat line per partition
    xr = nc.dram_tensor("xr", [128, 8 * S], bf16, kind="ExternalInput")
    xi = nc.dram_tensor("xi", [128, 8 * S], bf16, kind="ExternalInput")
    w = {
        nm: nc.dram_tensor(nm, [128, 8 * 256], bf16, kind="ExternalInput")
        for nm in ["wqr", "wqi", "wkr", "wki", "wor", "woi"]
    }
    wv = nc.dram_tensor("wv", [128, 8 * 512], bf16, kind="ExternalInput")
    bqk_d = nc.dram_tensor("bqk", [128, 8], f32, kind="ExternalInput")
    bo_d = nc.dram_tensor("bo", [128, 4], f32, kind="ExternalInput")
    cosd = nc.dram_tensor("cosd", [128, S], bf16, kind="ExternalInput")
    sind = nc.dram_tensor("sind", [128, S], bf16, kind="ExternalInput")
    identd = nc.dram_tensor("identd", [128, 128], bf16, kind="ExternalInput")
    o_r = nc.dram_tensor("o_r", [ODC, S], bf16, kind="ExternalOutput")
    o_i = nc.dram_tensor("o_i", [ODC, S], bf16, kind="ExternalOutput")

    def mm(out, lhsT, rhs, start, stop):
        nc.tensor.matmul(out, lhsT=lhsT, rhs=rhs, start=start, stop=stop)

    # packed score-column offsets per chunk: pk[qc][kvt], total PK[qc]
    pk, PK = [], []
    for qc in range(2):
        offs, run = [], 0
        for kvt in range((qc + 1) * 4):
            offs.append(run)
            lo = max(0, (kvt - 4 * qc) * 128)
            run += 512 - lo
        pk.append(offs)
        PK.append(run)

    with tile.TileContext(nc) as tc:
        with (
            tc.tile_pool(name="consts", bufs=1) as consts,
            tc.tile_pool(name="stage", bufs=3) as stage,
            tc.tile_pool(name="evp", bufs=3) as evp,
            tc.tile_pool(name="c1p", bufs=2) as c1p,
            tc.tile_pool(name="efp", bufs=4) as efp,
            tc.tile_pool(name="ebp", bufs=4) as ebp,
            tc.tile_pool(name="recp", bufs=4) as recp,
            tc.tile_pool(name="atp", bufs=8) as atp,
            tc.tile_pool(name="ttp", bufs=4) as ttp,
            tc.tile_pool(name="lop", bufs=4) as lop,
            tc.tile_pool(name="oop", bufs=2) as oop,
            tc.tile_pool(name="psA", bufs=3, space="PSUM") as psA,
            tc.tile_pool(name="psB", bufs=2, space="PSUM") as psB,
            tc.tile_pool(name="dram", bufs=1, space="DRAM") as dram,
        ):
            # ---- constants / persistent SBUF ----
            # x chunk-0 and the QK weights first: they gate the first matmul.
            x_sb = {}
            for key, src in (("r", xr), ("i", xi)):
                t = consts.tile([128, 8, S], bf16, tag=f"x{key}")
                rr = src.rearrange("p (kt m) -> p kt m", kt=8)
                nc.sync.dma_start(t[:, :, 0:512], rr[:, :, 0:512])
                x_sb[key] = t
            w_sb = {}
            for nm in ["wqr", "wqi", "wkr", "wki"]:
                t = consts.tile([128, 8, 256], bf16, tag=nm)
                nc.sync.dma_start(t, w[nm].rearrange("p (kt m) -> p kt m", kt=8))
                w_sb[nm] = t
            bqk_sb = consts.tile([128, 8], f32, tag="bqk")
            nc.sync.dma_start(bqk_sb, bqk_d[:, :])
            cos_sb = consts.tile([128, S], bf16, tag="cos")
            nc.sync.dma_start(cos_sb, cosd[:, :])
            sin_sb = consts.tile([128, S], bf16, tag="sin")
            nc.sync.dma_start(sin_sb, sind[:, :])
            wv_sb = consts.tile([128, 8, 512], bf16, tag="wv")
            nc.sync.dma_start(wv_sb, wv.rearrange("p (kt m) -> p kt m", kt=8))
            for key, src in (("r", xr), ("i", xi)):
                rr = src.rearrange("p (kt m) -> p kt m", kt=8)
                nc.sync.dma_start(x_sb[key][:, :, 512:1024], rr[:, :, 512:1024])

            # tiny warmup collective: absorbs the CC core's first-collective
            # startup cost (~11us) while the projections run.
            wagin = dram.tile([1, 4], f32, tag="wagin", name="wagin")
            wagout = dram.tile([4, 1, 4], f32, tag="wagout", name="wagout")
            wz = consts.tile([1, 4], f32, tag="wz")
            nc.vector.memset(wz, 0.0)
            nc.gpsimd.dma_start(wagin[:, :], wz)
            nc.gpsimd.collective_compute(
                "AllGather",
                mybir.AluOpType.bypass,
                replica_groups=groups,
                ins=[wagin[:].opt()],
                outs=[wagout[:].opt()],
            )

            eps_t = consts.tile([128, 1], f32, tag="eps")
            nc.vector.memset(eps_t, 1e-6 * float(gt) * float(gt))
            ident = consts.tile([128, 128], bf16, tag="ident")
            nc.sync.dma_start(ident, identd[:, :])
            bo_sb = consts.tile([128, 4], f32, tag="bo")
            nc.sync.dma_start(bo_sb, bo_d[:, :])
            for nm in ["wor", "woi"]:
                t = consts.tile([128, 8, 256], bf16, tag=nm)
                nc.sync.dma_start(t, w[nm].rearrange("p (kt m) -> p kt m", kt=8))
                w_sb[nm] = t

            Q = consts.tile([128, GH, S], bf16, tag="Q")
            K1 = consts.tile([128, GH, S], bf16, tag="K1")
            K2 = consts.tile([128, GH, S], bf16, tag="K2")
            # V combined per kv-tile/head: cols 0:64 = vr, 64:128 = vi, 128 = ones
            Vc = consts.tile([128, 8, GH, 129], bf16, tag="Vc")
            nc.vector.memset(Vc[:, :, :, 128:129], 1.0)

            # ---- phase 1: projections ----
            # (name, x key, w name, bias col base, rot targets, nr targets)
            # targets: (tensor, row0); ki additionally writes negated K2 rows.
            projs = [
                ("qr", "r", "wqr", 0, [(0, 0)], [(0, 32)]),
                ("qi", "i", "wqi", 2, [(0, 64)], [(0, 96)]),
                ("kr", "r", "wkr", 4, [(1, 0), (2, 64)], [(1, 32), (2, 96)]),
                ("ki", "i", "wki", 6, [(1, 64)], [(1, 96)]),
            ]
            qk_tensors = {0: Q, 1: K1, 2: K2}

            agin = [
                dram.tile([512, 512], bf16, tag=f"agin{qc}", name=f"agin{qc}")
                for qc in range(2)
            ]
            agout = [
                [
                    dram.tile(
                        [4, 256, 512], bf16, tag=f"agout{qc}_{ri}",
                        name=f"agout{qc}_{ri}",
                    )
                    for ri in range(2)
                ]
                for qc in range(2)
            ]
            gg = float(gt) * float(gt)

            def scores(qc, h, ef):
                """Score matmuls (kv tiles paired into 2-bank PSUM tiles) +
                one psi evacuation + one fused |z|^2 per pair.  The psi
                evacuation alternates between the scalar engine (as a
                table-free Square, feeding (a^2+b)*s) and the vector engine
                (plain copy, feeding (a^2+b^2)*s) to balance engine load."""
                nkv = (qc + 1) * 4
                for kv0 in range(0, nkv, 2):
                    psr2 = psA.tile([128, 1024], f32, tag="ps2")
                    psi2 = psA.tile([128, 1024], f32, tag="ps2")
                    col = 0
                    for kvt in (kv0, kv0 + 1):
                        lo = max(0, (kvt - 4 * qc) * 128)
                        N = 512 - lo
                        qsl = Q[:, h, qc * 512 + lo : (qc + 1) * 512]
                        ksl = slice(kvt * 128, (kvt + 1) * 128)
                        mm(
                            psr2[:, col : col + N], K1[:, h, ksl], qsl,
                            start=True, stop=True,
                        )
                        mm(
                            psi2[:, col : col + N], K2[:, h, ksl], qsl,
                            start=True, stop=True,
                        )
                        col += N
                    c1 = c1p.tile([128, 1024], f32, tag="c1")
                    if (kv0 // 2) % 2 == 0:
                        nc.scalar.activation(c1[:, :col], psi2[:, :col], AF.Square)
                        op, imm = sqadd, gg
                    else:
                        nc.vector.tensor_copy(c1[:, :col], psi2[:, :col])
                        op, imm = magsq, gg
                    nc.vector._custom_dve(
                        op,
                        out=ef[:, pk[qc][kv0] : pk[qc][kv0] + col],
                        in0=psr2[:, :col],
                        in1=c1[:, :col],
                        imm2=imm,
                    )

            def av(qc, h, eb, A):
                """AV with stationary score blocks -> [token, d] + normalise."""
                for t in range(4):
                    nkv_t = 4 * qc + t + 1
                    pd = psB.tile([128, 512], f32, tag="pav")
                    for kvt in range(nkv_t):
                        lo = max(0, (kvt - 4 * qc) * 128)
                        blk = pk[qc][kvt] + t * 128 - lo
                        mm(
                            pd[:, 0:129],
                            eb[:, blk : blk + 128],
                            Vc[:, kvt, h, :],
                            start=(kvt == 0),
                            stop=(kvt == nkv_t - 1),
                        )
                    rec = recp.tile([128, 1], f32, tag="rec")
                    nc.vector.reciprocal(rec, pd[:, 128:129])
                    # r cols -> A[t][:, h*64:...], i cols -> A[t][:, 256+h*64:...]
                    av_view = pd[:, 0:128].rearrange("p (x d) -> p x d", x=2)
                    out_view = A[t].rearrange("p (x hd) -> p x hd", x=2)[
                        :, :, h * 64 : (h + 1) * 64
                    ]
                    nc.vector.tensor_scalar_mul(out_view, av_view, rec)

            def attention(qc):
                A = [
                    atp.tile([128, 512], bf16, tag="A", name=f"A{qc}_{t}")
                    for t in range(4)
                ]
                efs, ebs = {}, {}
                for h in range(GH):
                    efs[h] = efp.tile([128, PK[1]], bf16, tag="ef", name=f"ef{qc}_{h}")
                    scores(qc, h, efs[h])
                for h in range(GH):
                    nc.scalar.activation(
                        efs[h][:, : PK[qc]],
                        efs[h][:, : PK[qc]],
                        AF.Sqrt,
                        bias=eps_t,
                    )
                for h in range(GH):
                    ebs[h] = ebp.tile([128, PK[1]], bf16, tag="eb", name=f"eb{qc}_{h}")
                    nc.scalar.activation(
                        ebs[h][:, : PK[qc]], efs[h][:, : PK[qc]], AF.Exp
                    )
                    for kvt in range(qc * 4, (qc + 1) * 4):
                        po = pk[qc][kvt]
                        nc.gpsimd.affine_select(
                            out=ebs[h][:, po : po + 128],
                            in_=ebs[h][:, po : po + 128],
                            compare_op=mybir.AluOpType.is_ge,
                            fill=0.0,
                            base=0,
                            channel_multiplier=-1,
                            pattern=[[1, 128]],
                        )

                # AVs for a head pair, then immediately transpose + stage +
                # AllGather that pair's rows (r and i) while the next pair's
                # AVs run.  agin rows: [r-h01 | i-h01 | r-h23 | i-h23].
                for hp in range(2):
                    av(qc, 2 * hp, ebs[2 * hp], A)
                    av(qc, 2 * hp + 1, ebs[2 * hp + 1], A)
                    for j, blk in enumerate((hp, 2 + hp)):
                        tb2 = ttp.tile(
                            [128, 512], bf16, tag="tb2", name=f"tb{qc}_{hp}_{j}"
                        )
                        for t in range(4):
                            pt = psB.tile([128, 1024], bf16, tag="pav")
                            nc.tensor.transpose(
                                pt[:, 0:128],
                                A[t][:, blk * 128 : (blk + 1) * 128],
                                ident,
                            )
                            nc.vector.tensor_copy(
                                tb2[:, t * 128 : (t + 1) * 128], pt[:, 0:128]
                            )
                        nc.gpsimd.dma_start(
                            agin[qc][hp * 256 + j * 128 : hp * 256 + (j + 1) * 128, :],
                            tb2,
                        )
                    rs = slice(hp * 256, (hp + 1) * 256)
                    nc.gpsimd.collective_compute(
                        "AllGather",
                        mybir.AluOpType.bypass,
                        replica_groups=groups,
                        ins=[agin[qc][rs, :].opt()],
                        outs=[agout[qc][hp][:].opt()],
                    )

            for c in range(2):
                csl = slice(c * 512, (c + 1) * 512)
                for pname, xkey, wname, bc, rot_tgts, nr_tgts in projs:
                    for mt in range(2):  # 0 = rot dims, 1 = non-rot dims
                        pst = psA.tile([128, 512], f32, tag="ps2")
                        for kt in range(8):
                            mm(
                                pst,
                                w_sb[wname][:, kt, mt * 128 : (mt + 1) * 128],
                                x_sb[xkey][:, kt, csl],
                                start=(kt == 0),
                                stop=(kt == 7),
                            )
                        bcol = bqk_sb[:, bc + mt : bc + mt + 1]
                        if mt == 0:
                            # bias-add during evacuation, then rope in bf16
                            tb = stage.tile([128, 512], bf16, tag="tb")
                            nc.scalar.activation(tb, pst, AF.Identity, bias=bcol)
                            shuf = stage.tile([128, 512], bf16, tag="shuf")
                            nc.vector.stream_shuffle(shuf, tb, mask=_PAIRSWAP)
                            nc.vector.tensor_mul(shuf, shuf, sin_sb[:, csl])
                            t2 = stage.tile([128, 512], bf16, tag="t2")
                            nc.vector.tensor_mul(t2, tb, cos_sb[:, csl])
                            nc.vector.tensor_add(t2, t2, shuf)
                            src_t = t2
                        else:
                            evn = evp.tile([128, 512], bf16, tag="ev")
                            nc.scalar.activation(evn, pst, AF.Identity, bias=bcol)
                            src_t = evn
                        tgts = rot_tgts if mt == 0 else nr_tgts
                        eng = nc.sync if mt == 0 else nc.scalar
                        for tid, row0 in tgts:
                            dst = qk_tensors[tid]
                            for h in range(GH):
                                eng.dma_start(
                                    dst[row0 : row0 + 32, h, csl],
                                    src_t[h * 32 : (h + 1) * 32, :],
                                )
                        if pname == "ki":  # negated copy into K2 rows 0:32/32:64
                            neg = evp.tile([128, 512], bf16, tag="ev")
                            nc.vector.tensor_scalar_mul(neg, src_t, -1.0)
                            row0 = 0 if mt == 0 else 32
                            for h in range(GH):
                                nc.sync.dma_start(
                                    K2[row0 : row0 + 32, h, csl],
                                    neg[h * 32 : (h + 1) * 32, :],
                                )

                # V: stationary x-block, moving [wvr|wvi] columns
                for tl in range(4):
                    tt = c * 4 + tl
                    tsl = slice(c * 512 + tl * 128, c * 512 + (tl + 1) * 128)
                    pv = psA.tile([128, 512], f32, tag="ps2")
                    for kt in range(8):
                        mm(
                            pv[:, 0:256],
                            x_sb["r"][:, kt, tsl],
                            wv_sb[:, kt, 0:256],
                            start=(kt == 0),
                            stop=(kt == 7),
                        )
                    for kt in range(8):
                        mm(
                            pv[:, 256:512],
                            x_sb["i"][:, kt, tsl],
                            wv_sb[:, kt, 256:512],
                            start=(kt == 0),
                            stop=(kt == 7),
                        )
                    ov = evp.tile([128, 512], bf16, tag="ov")
                    nc.scalar.copy(ov, pv)
                    nc.scalar.dma_start(
                        Vc[:, tt, :, 0:64],
                        ov[:, 0:256].rearrange("p (h d) -> p h d", h=GH),
                    )
                    nc.scalar.dma_start(
                        Vc[:, tt, :, 64:128],
                        ov[:, 256:512].rearrange("p (h d) -> p h d", h=GH),
                    )

            for qc in range(2):
                attention(qc)

            # ---- phase 3: output projections (AG(1) hides under O-proj(0)) ----
            for qc in range(2):
                for ri, wname, odst in ((0, "wor", o_r), (1, "woi", o_i)):
                    pos = [
                        psA.tile([128, 512], f32, tag="ps2", name=f"po{_i}")
                        for _i in range(2)
                    ]
                    for ht in range(8):
                        g, hp = ht // 2, ht % 2
                        lt = lop.tile([128, 512], bf16, tag="lt")
                        nc.sync.dma_start(
                            lt, agout[qc][hp][g, ri * 128 : (ri + 1) * 128, :]
                        )
                        for odt in range(2):
                            mm(
                                pos[odt],
                                w_sb[wname][:, ht, odt * 128 : (odt + 1) * 128],
                                lt,
                                start=(ht == 0),
                                stop=(ht == 7),
                            )
                    for odt in range(2):
                        oo = oop.tile([128, 512], bf16, tag="oo")
                        nc.scalar.activation(
                            oo,
                            pos[odt],
                            AF.Identity,
                            bias=bo_sb[:, 2 * ri + odt : 2 * ri + odt + 1],
                        )
                        nc.scalar.dma_start(
                            odst[
                                odt * 128 : (odt + 1) * 128,
                                qc * 512 : (qc + 1) * 512,
                            ],
                            oo,
                        )

    return nc


def _host_prep(inputs):
    """Fold ent/scale/bv on host; build per-core input maps (bf16 device layout)."""
    import ml_dtypes

    bf16 = ml_dtypes.bfloat16
    f = lambda x: np.asarray(x, dtype=np.float32)
    real, imag = f(inputs["real"]), f(inputs["imag"])
    ent = np.asarray(inputs["ent"], np.float64)
    scale = 1.0 / math.sqrt(HD)

    def fold_w(W, do_ent, sc=1.0):
        W = np.asarray(W, np.float64).reshape(DIM, HEADS, HD)
        if do_ent:
            W = np.einsum("chd,hx->cxd", W, ent)
        return W * sc  # [DIM, HEADS, HD] float64

    def fold_b(b, do_ent, sc=1.0):
        b = np.asarray(b, np.float64).reshape(HEADS, HD)
        if do_ent:
            b = np.einsum("hd,hx->xd", b, ent)
        return b * sc

    Wq_r = fold_w(inputs["Wq_r"], True, scale)
    Wq_i = fold_w(inputs["Wq_i"], True, scale)
    Wk_r = fold_w(inputs["Wk_r"], True)
    Wk_i = fold_w(inputs["Wk_i"], True)
    Wv_r = fold_w(inputs["Wv_r"], False)
    Wv_i = fold_w(inputs["Wv_i"], False)
    bq_r = fold_b(inputs["bq_r"], True, scale)
    bq_i = fold_b(inputs["bq_i"], True, scale)
    bk_r = fold_b(inputs["bk_r"], True)
    bk_i = fold_b(inputs["bk_i"], True)
    Wo_r = np.asarray(inputs["Wo_r"], np.float64)
    Wo_i = np.asarray(inputs["Wo_i"], np.float64)
    bo_r = np.asarray(inputs["bo_r"], np.float64) + np.asarray(
        inputs["bv_r"], np.float64
    ) @ Wo_r
    bo_i = np.asarray(inputs["bo_i"], np.float64) + np.asarray(
        inputs["bv_i"], np.float64
    ) @ Wo_i

    strength = float(np.asarray(inputs["strength"]).reshape(-1)[0])
    temp = float(np.asarray(inputs["temp"]).reshape(-1)[0])
    gt = (1.0 / (1.0 + math.exp(-strength))) / max(temp, 0.01)

    # rope tables in device layout: row h*32+d (d<32), freq j=d//2
    rot_freqs = np.asarray(inputs["rot_freqs"], np.float64)  # [16]
    pos = np.arange(S, dtype=np.float64)
    emb = pos[:, None] * rot_freqs[None, :]  # [S, 16]
    cos_t = np.cos(emb)
    sin_t = np.sin(emb)
    cosd = np.empty((128, S), np.float32)
    sind = np.empty((128, S), np.float32)
    for hh in range(4):
        for d in range(32):
            r = hh * 32 + d
            cosd[r] = cos_t[:, d // 2]
            sind[r] = (-sin_t if d % 2 == 0 else sin_t)[:, d // 2]

    def tile_kp(Wdev):
        # [DIM, M] -> [128, 8*M] with (kp, kt, m) = W[kt*128+kp, m]
        M = Wdev.shape[1]
        return np.ascontiguousarray(
            Wdev.reshape(8, 128, M).transpose(1, 0, 2).reshape(128, 8 * M)
        )

    def qk_dev(Wf, bf_, g):
        # [DIM,H,HD]/[H,HD] -> per-core [DIM,256]/[256] in [rot x 4h | nr x 4h]
        hs = slice(g * GH, (g + 1) * GH)
        Wc, bc = Wf[:, hs, :], bf_[hs, :]
        wd = np.concatenate(
            [
                Wc[:, :, :ROTD].reshape(DIM, GH * ROTD),
                Wc[:, :, ROTD:].reshape(DIM, GH * ROTD),
            ],
            axis=1,
        )
        bd = np.concatenate(
            [bc[:, :ROTD].reshape(GH * ROTD), bc[:, ROTD:].reshape(GH * ROTD)]
        )
        return wd, bd

    ident = np.eye(128, dtype=np.float32)

    in_maps = []
    for core in range(NCORES):
        b, g = core // 4, core % 4
        hs = slice(g * GH, (g + 1) * GH)
        xr_dev = np.ascontiguousarray(
            real[b].T.reshape(8, 128, S).transpose(1, 0, 2).reshape(128, 8 * S)
        )
        xi_dev = np.ascontiguousarray(
            imag[b].T.reshape(8, 128, S).transpose(1, 0, 2).reshape(128, 8 * S)
        )
        wv_dev = np.concatenate(
            [Wv_r[:, hs, :].reshape(DIM, 256), Wv_i[:, hs, :].reshape(DIM, 256)],
            axis=1,
        )
        bqk = np.empty((128, 8), np.float32)
        m = {
            "xr": xr_dev.astype(bf16),
            "xi": xi_dev.astype(bf16),
            "cosd": cosd.astype(bf16),
            "sind": sind.astype(bf16),
            "identd": ident.astype(bf16),
            "wv": tile_kp(wv_dev).astype(bf16),
            "wor": tile_kp(Wo_r[:, g * ODC : (g + 1) * ODC]).astype(bf16),
            "woi": tile_kp(Wo_i[:, g * ODC : (g + 1) * ODC]).astype(bf16),
        }
        for j, (nm, Wf, bf_) in enumerate(
            (
                ("qr", Wq_r, bq_r),
                ("qi", Wq_i, bq_i),
                ("kr", Wk_r, bk_r),
                ("ki", Wk_i, bk_i),
            )
        ):
            wd, bd = qk_dev(Wf, bf_, g)
            m["w" + nm] = tile_kp(wd).astype(bf16)
            bqk[:, 2 * j] = bd[0:128]
            bqk[:, 2 * j + 1] = bd[128:256]
        m["bqk"] = bqk
        bo = np.empty((128, 4), np.float32)
        bo[:, 0] = bo_r[g * ODC : g * ODC + 128]
        bo[:, 1] = bo_r[g * ODC + 128 : g * ODC + 256]
        bo[:, 2] = bo_i[g * ODC : g * ODC + 128]
        bo[:, 3] = bo_i[g * ODC + 128 : g * ODC + 256]
        m["bo"] = bo
        in_maps.append(m)
    return in_maps, gt


def kernel(**inputs):
    from concourse import bass_utils

    in_maps, gt = _host_prep(inputs)
    nc = _build(gt)
    nc.finalize()
    res = bass_utils.run_bass_kernel_spmd(nc, in_maps, core_ids=list(range(NCORES)))
    out_r = np.empty((B, S, DIM), np.float32)
    out_i = np.empty((B, S, DIM), np.float32)
    for core in range(NCORES):
        b, g = core // 4, core % 4
        out_r[b, :, g * ODC : (g + 1) * ODC] = (
            np.asarray(res.results[core]["o_r"]).astype(np.float32).T
        )
        out_i[b, :, g * ODC : (g + 1) * ODC] = (
            np.asarray(res.results[core]["o_i"]).astype(np.float32).T
        )
    return np.stack([out_r, out_i], axis=0)


# revision 57
# speedup vs baseline: 1.1081x; 1.0165x over previous
"""Trainium2 Bass kernel for the EntangledInterferenceLayer problem.

Math transformations done on host (numpy), all exact up to fp rounding:
  * The HxH entanglement mix commutes with RoPE (cos/sin are head-independent),
    so it folds into the Q/K projection weights + biases.
  * The per-head phase shift rotates q and k by the same complex phase, and the
    attention logits use q * conj(k) -> the phase cancels exactly.  Dropped.
  * 1/sqrt(head_dim) folds into the Q weights/bias.
  * The V-projection bias contributes bv @ Wo to every output row (softmax rows
    sum to 1), so it folds into the output bias.

Sharding (8 cores): core = (batch b, head-group g of 4 heads). Each core
projects Q/K/V for its heads, runs causal complex-magnitude attention, then an
AllGather of attention outputs within the 4-core batch group lets every core
compute a 256-column slice of both output projections.

Device-side design (v2, bf16):
  * All matmul operands bf16 (2x stream rate vs fp32r at the ramped PE clock);
    PSUM accumulation stays f32.  Weights and x are pre-tiled on host into
    SBUF layout [128, kt, m] so every load is one fat DMA, loaded once.
  * Q/K computed transposed ([complex-component-row, token]); per head the 128
    contraction rows are [qr-rot, qr-nr, qi-rot, qi-nr]; K2 = [-ki, kr] gives
    imaginary logits with plain matmuls.
  * Projection biases are folded into the PSUM->SBUF evacuation via the ACT
    engine's per-partition bias operand (no bias matmuls).
  * Scores are built [kv, q] with causally-trimmed streams, packed contiguously
    per (chunk, head) so Sqrt and Exp run as one big ACT op each (table loads
    batched pairwise across heads).
  * AV uses the score tile as the *stationary* operand producing [token, d]
    blocks; V real/imag/ones are one [128,129] moving operand, so softmax
    denominators land per-partition: reciprocal is a [128,1] DVE op and the
    normalisation folds into a single strided tensor_scalar_mul.
  * Normalised outputs are transposed back to [head-dim, token] with PE
    transposes (identity matmul), staged to DRAM, and AllGathered per chunk so
    the collective overlaps the next chunk's attention / output projection.
"""

import math

import numpy as np

B, S, DIM = 2, 1024, 1024
HEADS, HD, ROTD = 16, 64, 32
GH = 4  # heads per core
ODC = 256  # out-dim columns per core
NCORES = 8

_PAIRSWAP = [i ^ 1 for i in range(32)]


def _register_dve_op(name, spec_builder):
    """Register a fused custom DVE op (idempotent)."""
    from concourse import dve_ops as DO
    from concourse.dve_spec import lower

    if name in DO._SUB_OPCODE_FOR_NAME:
        return next(o for o in DO.OPS if o.name == name)
    spec = spec_builder()
    opcode = DO._CUSTOM_DVE_ROW_BASE + len(DO.OPS)
    DO._SUB_OPCODE_FOR_NAME[name] = opcode
    shas = {}
    for ver in ("v3", "v4"):
        try:
            s = DO.DveOpSpec(
                name=name, opcode=opcode, uops=lower(spec, ver=ver), rd1_en=True
            )
            shas[ver] = s.sha(ver)
        except Exception:
            pass
    op = DO.DveOp(name, spec, subdim=False, uops_sha=shas)
    DO.OPS.append(op)
    DO.CUSTOM_DVE_SPECS[name] = spec
    return op


def _register_magsq():
    """out = (in0^2 + in1^2) * imm2"""
    import numpy as np
    from concourse.dve_spec import Spec, Src0, Src1, C2, sq

    return _register_dve_op(
        "ANT_MAGSQ",
        lambda: Spec(
            body=(sq(Src0) + sq(Src1)) * C2,
            reference=lambda in0, in1, s0, s1, imm2: (
                in0.astype(np.float32) ** 2 + in1.astype(np.float32) ** 2
            )
            * np.float32(imm2),
        ),
    )


def _register_sqadd():
    """out = (in0^2 + in1) * imm2 — in1 is a pre-squared operand."""
    import numpy as np
    from concourse.dve_spec import Spec, Src0, Src1, C2, sq

    return _register_dve_op(
        "ANT_SQADD",
        lambda: Spec(
            body=(sq(Src0) + Src1) * C2,
            reference=lambda in0, in1, s0, s1, imm2: (
                in0.astype(np.float32) ** 2 + in1.astype(np.float32)
            )
            * np.float32(imm2),
        ),
    )


def _build(gt: float, groups=None):
    import concourse.mybir as mybir
    import concourse.tile as tile
    from concourse import bacc

    f32 = mybir.dt.float32
    bf16 = mybir.dt.bfloat16
    AF = mybir.ActivationFunctionType
    magsq = _register_magsq()
    sqadd = _register_sqadd()

    nc = bacc.Bacc("TRN2", target_bir_lowering=False, num_devices=NCORES)
    if groups is None:
        groups = [[0, 1, 2, 3], [4, 5, 6, 7]]

    # host-pre-tiled inputs: [128, kt*m] so DMAs are one fat line per partition
    xr = nc.dram_tensor("xr", [128, 8 * S], bf16, kind="ExternalInput")
    xi = nc.dram_tensor("xi", [128, 8 * S], bf16, kind="ExternalInput")
    w = {
        nm: nc.dram_tensor(nm, [128, 8 * 256], bf16, kind="ExternalInput")
        for nm in ["wqr", "wqi", "wkr", "wki", "wor", "woi"]
    }
    wv = nc.dram_tensor("wv", [128, 8 * 512], bf16, kind="ExternalInput")
    bqk_d = nc.dram_tensor("bqk", [128, 8], f32, kind="ExternalInput")
    bo_d = nc.dram_tensor("bo", [128, 4], f32, kind="ExternalInput")
    cosd = nc.dram_tensor("cosd", [128, S], bf16, kind="ExternalInput")
    sind = nc.dram_tensor("sind", [128, S], bf16, kind="ExternalInput")
    identd = nc.dram_tensor("identd", [128, 128], bf16, kind="ExternalInput")
    o_r = nc.dram_tensor("o_r", [ODC, S], bf16, kind="ExternalOutput")
    o_i = nc.dram_tensor("o_i", [ODC, S], bf16, kind="ExternalOutput")

    def mm(out, lhsT, rhs, start, stop):
        nc.tensor.matmul(out, lhsT=lhsT, rhs=rhs, start=start, stop=stop)

    # packed score-column offsets per chunk: pk[qc][kvt], total PK[qc]
    pk, PK = [], []
    for qc in range(2):
        offs, run = [], 0
        for kvt in range((qc + 1) * 4):
            offs.append(run)
            lo = max(0, (kvt - 4 * qc) * 128)
            run += 512 - lo
        pk.append(offs)
        PK.append(run)

    with tile.TileContext(nc) as tc:
        with (
            tc.tile_pool(name="consts", bufs=1) as consts,
            tc.tile_pool(name="stage", bufs=3) as stage,
            tc.tile_pool(name="evp", bufs=3) as evp,
            tc.tile_pool(name="c1p", bufs=2) as c1p,
            tc.tile_pool(name="efp", bufs=4) as efp,
            tc.tile_pool(name="ebp", bufs=4) as ebp,
            tc.tile_pool(name="recp", bufs=4) as recp,
            tc.tile_pool(name="atp", bufs=8) as atp,
            tc.tile_pool(name="ttp", bufs=4) as ttp,
            tc.tile_pool(name="lop", bufs=4) as lop,
            tc.tile_pool(name="oop", bufs=2) as oop,
            tc.tile_pool(name="psA", bufs=3, space="PSUM") as psA,
            tc.tile_pool(name="psB", bufs=2, space="PSUM") as psB,
            tc.tile_pool(name="dram", bufs=1, space="DRAM") as dram,
        ):
            # ---- constants / persistent SBUF ----
            # x chunk-0 and the QK weights first: they gate the first matmul.
            x_sb = {}
            for key, src in (("r", xr), ("i", xi)):
                t = consts.tile([128, 8, S], bf16, tag=f"x{key}")
                rr = src.rearrange("p (kt m) -> p kt m", kt=8)
                # per-kt pieces: the first projection group's MMs start as
                # soon as each contraction slice lands
                for kt in range(8):
                    nc.sync.dma_start(t[:, kt, 0:512], rr[:, kt, 0:512])
                x_sb[key] = t
            w_sb = {}
            for nm in ["wqr", "wqi", "wkr", "wki"]:
                t = consts.tile([128, 8, 256], bf16, tag=nm)
                nc.sync.dma_start(t, w[nm].rearrange("p (kt m) -> p kt m", kt=8))
                w_sb[nm] = t
            bqk_sb = consts.tile([128, 8], f32, tag="bqk")
            nc.sync.dma_start(bqk_sb, bqk_d[:, :])
            cos_sb = consts.tile([128, S], bf16, tag="cos")
            nc.sync.dma_start(cos_sb, cosd[:, :])
            sin_sb = consts.tile([128, S], bf16, tag="sin")
            nc.sync.dma_start(sin_sb, sind[:, :])
            wv_sb = consts.tile([128, 8, 512], bf16, tag="wv")
            nc.sync.dma_start(wv_sb, wv.rearrange("p (kt m) -> p kt m", kt=8))
            for key, src in (("r", xr), ("i", xi)):
                rr = src.rearrange("p (kt m) -> p kt m", kt=8)
                nc.sync.dma_start(x_sb[key][:, :, 512:1024], rr[:, :, 512:1024])

            # tiny warmup collective: absorbs the CC core's first-collective
            # startup cost (~11us) while the projections run.
            wagin = dram.tile([1, 4], f32, tag="wagin", name="wagin")
            wagout = dram.tile([4, 1, 4], f32, tag="wagout", name="wagout")
            wz = consts.tile([1, 4], f32, tag="wz")
            nc.vector.memset(wz, 0.0)
            nc.gpsimd.dma_start(wagin[:, :], wz)
            nc.gpsimd.collective_compute(
                "AllGather",
                mybir.AluOpType.bypass,
                replica_groups=groups,
                ins=[wagin[:].opt()],
                outs=[wagout[:].opt()],
            )

            eps_t = consts.tile([128, 1], f32, tag="eps")
            nc.vector.memset(eps_t, 1e-6 * float(gt) * float(gt))
            ident = consts.tile([128, 128], bf16, tag="ident")
            nc.sync.dma_start(ident, identd[:, :])
            bo_sb = consts.tile([128, 4], f32, tag="bo")
            nc.sync.dma_start(bo_sb, bo_d[:, :])
            for nm in ["wor", "woi"]:
                t = consts.tile([128, 8, 256], bf16, tag=nm)
                nc.sync.dma_start(t, w[nm].rearrange("p (kt m) -> p kt m", kt=8))
                w_sb[nm] = t

            Q = consts.tile([128, GH, S], bf16, tag="Q")
            K1 = consts.tile([128, GH, S], bf16, tag="K1")
            K2 = consts.tile([128, GH, S], bf16, tag="K2")
            # V combined per kv-tile/head: cols 0:64 = vr, 64:128 = vi, 128 = ones
            Vc = consts.tile([128, 8, GH, 129], bf16, tag="Vc")
            nc.vector.memset(Vc[:, :, :, 128:129], 1.0)

            # ---- phase 1: projections ----
            # (name, x key, w name, bias col base, rot targets, nr targets)
            # targets: (tensor, row0); ki additionally writes negated K2 rows.
            projs = [
                ("qr", "r", "wqr", 0, [(0, 0)], [(0, 32)]),
                ("qi", "i", "wqi", 2, [(0, 64)], [(0, 96)]),
                ("kr", "r", "wkr", 4, [(1, 0), (2, 64)], [(1, 32), (2, 96)]),
                ("ki", "i", "wki", 6, [(1, 64)], [(1, 96)]),
            ]
            qk_tensors = {0: Q, 1: K1, 2: K2}

            agin = [
                dram.tile([512, 512], bf16, tag=f"agin{qc}", name=f"agin{qc}")
                for qc in range(2)
            ]
            agout = [
                [
                    dram.tile(
                        [4, 256, 512], bf16, tag=f"agout{qc}_{ri}",
                        name=f"agout{qc}_{ri}",
                    )
                    for ri in range(2)
                ]
                for qc in range(2)
            ]
            gg = float(gt) * float(gt)

            def scores(qc, h, ef):
                """Score matmuls (kv tiles paired into 2-bank PSUM tiles) +
                one psi evacuation + one fused |z|^2 per pair.  The psi
                evacuation alternates between the scalar engine (as a
                table-free Square, feeding (a^2+b)*s) and the vector engine
                (plain copy, feeding (a^2+b^2)*s) to balance engine load."""
                nkv = (qc + 1) * 4
                for kv0 in range(0, nkv, 2):
                    psr2 = psA.tile([128, 1024], f32, tag="ps2")
                    psi2 = psA.tile([128, 1024], f32, tag="ps2")
                    col = 0
                    for kvt in (kv0, kv0 + 1):
                        lo = max(0, (kvt - 4 * qc) * 128)
                        N = 512 - lo
                        qsl = Q[:, h, qc * 512 + lo : (qc + 1) * 512]
                        ksl = slice(kvt * 128, (kvt + 1) * 128)
                        mm(
                            psr2[:, col : col + N], K1[:, h, ksl], qsl,
                            start=True, stop=True,
                        )
                        mm(
                            psi2[:, col : col + N], K2[:, h, ksl], qsl,
                            start=True, stop=True,
                        )
                        col += N
                    c1 = c1p.tile([128, 1024], f32, tag="c1")
                    nc.vector.tensor_copy(c1[:, :col], psi2[:, :col])
                    nc.vector._custom_dve(
                        magsq,
                        out=ef[:, pk[qc][kv0] : pk[qc][kv0] + col],
                        in0=psr2[:, :col],
                        in1=c1[:, :col],
                        imm2=gg,
                    )

            def av(qc, h, eb, A):
                """AV with stationary score blocks -> [token, d] + normalise."""
                for t in range(4):
                    nkv_t = 4 * qc + t + 1
                    pd = psB.tile([128, 512], f32, tag="pav")
                    for kvt in range(nkv_t):
                        lo = max(0, (kvt - 4 * qc) * 128)
                        blk = pk[qc][kvt] + t * 128 - lo
                        mm(
                            pd[:, 0:129],
                            eb[:, blk : blk + 128],
                            Vc[:, kvt, h, :],
                            start=(kvt == 0),
                            stop=(kvt == nkv_t - 1),
                        )
                    rec = recp.tile([128, 1], f32, tag="rec")
                    nc.vector.reciprocal(rec, pd[:, 128:129])
                    # r cols -> A[t][:, h*64:...], i cols -> A[t][:, 256+h*64:...]
                    av_view = pd[:, 0:128].rearrange("p (x d) -> p x d", x=2)
                    out_view = A[t].rearrange("p (x hd) -> p x hd", x=2)[
                        :, :, h * 64 : (h + 1) * 64
                    ]
                    nc.vector.tensor_scalar_mul(out_view, av_view, rec)

            def attention(qc):
                A = [
                    atp.tile([128, 512], bf16, tag="A", name=f"A{qc}_{t}")
                    for t in range(4)
                ]
                efs, ebs = {}, {}
                for h in range(GH):
                    efs[h] = efp.tile([128, PK[1]], bf16, tag="ef", name=f"ef{qc}_{h}")
                    scores(qc, h, efs[h])
                for h in range(GH):
                    nc.scalar.activation(
                        efs[h][:, : PK[qc]],
                        efs[h][:, : PK[qc]],
                        AF.Sqrt,
                        bias=eps_t,
                    )
                for h in range(GH):
                    ebs[h] = ebp.tile([128, PK[1]], bf16, tag="eb", name=f"eb{qc}_{h}")
                    nc.scalar.activation(
                        ebs[h][:, : PK[qc]], efs[h][:, : PK[qc]], AF.Exp
                    )
                    for kvt in range(qc * 4, (qc + 1) * 4):
                        po = pk[qc][kvt]
                        nc.gpsimd.affine_select(
                            out=ebs[h][:, po : po + 128],
                            in_=ebs[h][:, po : po + 128],
                            compare_op=mybir.AluOpType.is_ge,
                            fill=0.0,
                            base=0,
                            channel_multiplier=-1,
                            pattern=[[1, 128]],
                        )

                # AVs for a head pair, then immediately transpose + stage +
                # AllGather that pair's rows (r and i) while the next pair's
                # AVs run.  agin rows: [r-h01 | i-h01 | r-h23 | i-h23].
                for hp in range(2):
                    av(qc, 2 * hp, ebs[2 * hp], A)
                    av(qc, 2 * hp + 1, ebs[2 * hp + 1], A)
                    for j, blk in enumerate((hp, 2 + hp)):
                        tb2 = ttp.tile(
                            [128, 512], bf16, tag="tb2", name=f"tb{qc}_{hp}_{j}"
                        )
                        for t in range(4):
                            pt = psB.tile([128, 1024], bf16, tag="pav")
                            nc.tensor.transpose(
                                pt[:, 0:128],
                                A[t][:, blk * 128 : (blk + 1) * 128],
                                ident,
                            )
                            nc.vector.tensor_copy(
                                tb2[:, t * 128 : (t + 1) * 128], pt[:, 0:128]
                            )
                        nc.gpsimd.dma_start(
                            agin[qc][hp * 256 + j * 128 : hp * 256 + (j + 1) * 128, :],
                            tb2,
                        )
                    rs = slice(hp * 256, (hp + 1) * 256)
                    nc.gpsimd.collective_compute(
                        "AllGather",
                        mybir.AluOpType.bypass,
                        replica_groups=groups,
                        ins=[agin[qc][rs, :].opt()],
                        outs=[agout[qc][hp][:].opt()],
                    )

            for c in range(2):
                csl = slice(c * 512, (c + 1) * 512)
                for pname, xkey, wname, bc, rot_tgts, nr_tgts in projs:
                    for mt in range(2):  # 0 = rot dims, 1 = non-rot dims
                        pst = psA.tile([128, 512], f32, tag="ps2")
                        for kt in range(8):
                            mm(
                                pst,
                                w_sb[wname][:, kt, mt * 128 : (mt + 1) * 128],
                                x_sb[xkey][:, kt, csl],
                                start=(kt == 0),
                                stop=(kt == 7),
                            )
                        bcol = bqk_sb[:, bc + mt : bc + mt + 1]
                        if mt == 0:
                            # bias-add during evacuation, then rope in bf16
                            tb = stage.tile([128, 512], bf16, tag="tb")
                            nc.scalar.activation(tb, pst, AF.Identity, bias=bcol)
                            shuf = stage.tile([128, 512], bf16, tag="shuf")
                            nc.vector.stream_shuffle(shuf, tb, mask=_PAIRSWAP)
                            nc.vector.tensor_mul(shuf, shuf, sin_sb[:, csl])
                            t2 = stage.tile([128, 512], bf16, tag="t2")
                            nc.vector.tensor_mul(t2, tb, cos_sb[:, csl])
                            nc.vector.tensor_add(t2, t2, shuf)
                            src_t = t2
                        else:
                            evn = evp.tile([128, 512], bf16, tag="ev")
                            nc.scalar.activation(evn, pst, AF.Identity, bias=bcol)
                            src_t = evn
                        tgts = rot_tgts if mt == 0 else nr_tgts
                        eng = nc.sync if mt == 0 else nc.scalar
                        for tid, row0 in tgts:
                            dst = qk_tensors[tid]
                            for h in range(GH):
                                eng.dma_start(
                                    dst[row0 : row0 + 32, h, csl],
                                    src_t[h * 32 : (h + 1) * 32, :],
                                )
                        if pname == "ki":  # negated copy into K2 rows 0:32/32:64
                            neg = evp.tile([128, 512], bf16, tag="ev")
                            nc.vector.tensor_scalar_mul(neg, src_t, -1.0)
                            row0 = 0 if mt == 0 else 32
                            for h in range(GH):
                                nc.sync.dma_start(
                                    K2[row0 : row0 + 32, h, csl],
                                    neg[h * 32 : (h + 1) * 32, :],
                                )

                # V: stationary x-block, moving [wvr|wvi] columns
                for tl in range(4):
                    tt = c * 4 + tl
                    tsl = slice(c * 512 + tl * 128, c * 512 + (tl + 1) * 128)
                    pv = psA.tile([128, 512], f32, tag="ps2")
                    for kt in range(8):
                        mm(
                            pv[:, 0:256],
                            x_sb["r"][:, kt, tsl],
                            wv_sb[:, kt, 0:256],
                            start=(kt == 0),
                            stop=(kt == 7),
                        )
                    for kt in range(8):
                        mm(
                            pv[:, 256:512],
                            x_sb["i"][:, kt, tsl],
                            wv_sb[:, kt, 256:512],
                            start=(kt == 0),
                            stop=(kt == 7),
                        )
                    ov = evp.tile([128, 512], bf16, tag="ov")
                    nc.scalar.copy(ov, pv)
                    nc.scalar.dma_start(
                        Vc[:, tt, :, 0:64],
                        ov[:, 0:256].rearrange("p (h d) -> p h d", h=GH),
                    )
                    nc.scalar.dma_start(
                        Vc[:, tt, :, 64:128],
                        ov[:, 256:512].rearrange("p (h d) -> p h d", h=GH),
                    )

            for qc in range(2):
                attention(qc)

            # ---- phase 3: output projections (AG(1) hides under O-proj(0)) ----
            for qc in range(2):
                for ri, wname, odst in ((0, "wor", o_r), (1, "woi", o_i)):
                    pos = [
                        psA.tile([128, 512], f32, tag="ps2", name=f"po{_i}")
                        for _i in range(2)
                    ]
                    for ht in range(8):
                        g, hp = ht // 2, ht % 2
                        lt = lop.tile([128, 512], bf16, tag="lt")
                        nc.sync.dma_start(
                            lt, agout[qc][hp][g, ri * 128 : (ri + 1) * 128, :]
                        )
                        for odt in range(2):
                            mm(
                                pos[odt],
                                w_sb[wname][:, ht, odt * 128 : (odt + 1) * 128],
                                lt,
                                start=(ht == 0),
                                stop=(ht == 7),
                            )
                    for odt in range(2):
                        oo = oop.tile([128, 512], bf16, tag="oo")
                        nc.scalar.activation(
                            oo,
                            pos[odt],
                            AF.Identity,
                            bias=bo_sb[:, 2 * ri + odt : 2 * ri + odt + 1],
                        )
                        nc.scalar.dma_start(
                            odst[
                                odt * 128 : (odt + 1) * 128,
                                qc * 512 : (qc + 1) * 512,
                            ],
                            oo,
                        )

    return nc


def _host_prep(inputs):
    """Fold ent/scale/bv on host; build per-core input maps (bf16 device layout)."""
    import ml_dtypes

    bf16 = ml_dtypes.bfloat16
    f = lambda x: np.asarray(x, dtype=np.float32)
    real, imag = f(inputs["real"]), f(inputs["imag"])
    ent = np.asarray(inputs["ent"], np.float64)
    scale = 1.0 / math.sqrt(HD)

    def fold_w(W, do_ent, sc=1.0):
        W = np.asarray(W, np.float64).reshape(DIM, HEADS, HD)
        if do_ent:
            W = np.einsum("chd,hx->cxd", W, ent)
        return W * sc  # [DIM, HEADS, HD] float64

    def fold_b(b, do_ent, sc=1.0):
        b = np.asarray(b, np.float64).reshape(HEADS, HD)
        if do_ent:
            b = np.einsum("hd,hx->xd", b, ent)
        return b * sc

    Wq_r = fold_w(inputs["Wq_r"], True, scale)
    Wq_i = fold_w(inputs["Wq_i"], True, scale)
    Wk_r = fold_w(inputs["Wk_r"], True)
    Wk_i = fold_w(inputs["Wk_i"], True)
    Wv_r = fold_w(inputs["Wv_r"], False)
    Wv_i = fold_w(inputs["Wv_i"], False)
    bq_r = fold_b(inputs["bq_r"], True, scale)
    bq_i = fold_b(inputs["bq_i"], True, scale)
    bk_r = fold_b(inputs["bk_r"], True)
    bk_i = fold_b(inputs["bk_i"], True)
    Wo_r = np.asarray(inputs["Wo_r"], np.float64)
    Wo_i = np.asarray(inputs["Wo_i"], np.float64)
    bo_r = np.asarray(inputs["bo_r"], np.float64) + np.asarray(
        inputs["bv_r"], np.float64
    ) @ Wo_r
    bo_i = np.asarray(inputs["bo_i"], np.float64) + np.asarray(
        inputs["bv_i"], np.float64
    ) @ Wo_i

    strength = float(np.asarray(inputs["strength"]).reshape(-1)[0])
    temp = float(np.asarray(inputs["temp"]).reshape(-1)[0])
    gt = (1.0 / (1.0 + math.exp(-strength))) / max(temp, 0.01)

    # rope tables in device layout: row h*32+d (d<32), freq j=d//2
    rot_freqs = np.asarray(inputs["rot_freqs"], np.float64)  # [16]
    pos = np.arange(S, dtype=np.float64)
    emb = pos[:, None] * rot_freqs[None, :]  # [S, 16]
    cos_t = np.cos(emb)
    sin_t = np.sin(emb)
    cosd = np.empty((128, S), np.float32)
    sind = np.empty((128, S), np.float32)
    for hh in range(4):
        for d in range(32):
            r = hh * 32 + d
            cosd[r] = cos_t[:, d // 2]
            sind[r] = (-sin_t if d % 2 == 0 else sin_t)[:, d // 2]

    def tile_kp(Wdev):
        # [DIM, M] -> [128, 8*M] with (kp, kt, m) = W[kt*128+kp, m]
        M = Wdev.shape[1]
        return np.ascontiguousarray(
            Wdev.reshape(8, 128, M).transpose(1, 0, 2).reshape(128, 8 * M)
        )

    def qk_dev(Wf, bf_, g):
        # [DIM,H,HD]/[H,HD] -> per-core [DIM,256]/[256] in [rot x 4h | nr x 4h]
        hs = slice(g * GH, (g + 1) * GH)
        Wc, bc = Wf[:, hs, :], bf_[hs, :]
        wd = np.concatenate(
            [
                Wc[:, :, :ROTD].reshape(DIM, GH * ROTD),
                Wc[:, :, ROTD:].reshape(DIM, GH * ROTD),
            ],
            axis=1,
        )
        bd = np.concatenate(
            [bc[:, :ROTD].reshape(GH * ROTD), bc[:, ROTD:].reshape(GH * ROTD)]
        )
        return wd, bd

    ident = np.eye(128, dtype=np.float32)

    in_maps = []
    for core in range(NCORES):
        b, g = core // 4, core % 4
        hs = slice(g * GH, (g + 1) * GH)
        xr_dev = np.ascontiguousarray(
            real[b].T.reshape(8, 128, S).transpose(1, 0, 2).reshape(128, 8 * S)
        )
        xi_dev = np.ascontiguousarray(
            imag[b].T.reshape(8, 128, S).transpose(1, 0, 2).reshape(128, 8 * S)
        )
        wv_dev = np.concatenate(
            [Wv_r[:, hs, :].reshape(DIM, 256), Wv_i[:, hs, :].reshape(DIM, 256)],
            axis=1,
        )
        bqk = np.empty((128, 8), np.float32)
        m = {
            "xr": xr_dev.astype(bf16),
            "xi": xi_dev.astype(bf16),
            "cosd": cosd.astype(bf16),
            "sind": sind.astype(bf16),
            "identd": ident.astype(bf16),
            "wv": tile_kp(wv_dev).astype(bf16),
            "wor": tile_kp(Wo_r[:, g * ODC : (g + 1) * ODC]).astype(bf16),
            "woi": tile_kp(Wo_i[:, g * ODC : (g + 1) * ODC]).astype(bf16),
        }
        for j, (nm, Wf, bf_) in enumerate(
            (
                ("qr", Wq_r, bq_r),
                ("qi", Wq_i, bq_i),
                ("kr", Wk_r, bk_r),
                ("ki", Wk_i, bk_i),
            )
        ):
            wd, bd = qk_dev(Wf, bf_, g)
            m["w" + nm] = tile_kp(wd).astype(bf16)
            bqk[:, 2 * j] = bd[0:128]
            bqk[:, 2 * j + 1] = bd[128:256]
        m["bqk"] = bqk
        bo = np.empty((128, 4), np.float32)
        bo[:, 0] = bo_r[g * ODC : g * ODC + 128]
        bo[:, 1] = bo_r[g * ODC + 128 : g * ODC + 256]
        bo[:, 2] = bo_i[g * ODC : g * ODC + 128]
        bo[:, 3] = bo_i[g * ODC + 128 : g * ODC + 256]
        m["bo"] = bo
        in_maps.append(m)
    return in_maps, gt


def kernel(**inputs):
    from concourse import bass_utils

    in_maps, gt = _host_prep(inputs)
    nc = _build(gt)
    nc.finalize()
    res = bass_utils.run_bass_kernel_spmd(nc, in_maps, core_ids=list(range(NCORES)))
    out_r = np.empty((B, S, DIM), np.float32)
    out_i = np.empty((B, S, DIM), np.float32)
    for core in range(NCORES):
        b, g = core // 4, core % 4
        out_r[b, :, g * ODC : (g + 1) * ODC] = (
            np.asarray(res.results[core]["o_r"]).astype(np.float32).T
        )
        out_i[b, :, g * ODC : (g + 1) * ODC] = (
            np.asarray(res.results[core]["o_i"]).astype(np.float32).T
        )
    return np.stack([out_r, out_i], axis=0)
